# revision 25
# baseline (speedup 1.0000x reference)
"""Causal attention (B=4, S=2048, D=1024) on 8 trn2 NeuronCores.

Sharding: 2 cores per batch element, split over KEYS (interleaved 128-row
key blocks: core parity 0 takes even blocks, parity 1 takes odd blocks).
Each core computes Q for all 2048 positions, K/V for its 1024 keys, the
causally-masked exp-score block-band, the unnormalized partial output
O_part = sum_k exp(s_qk) v_k and the partial softmax denominator
sums_q = sum_k exp(s_qk).  The host merges: O = (O_A + O_B)/(sums_A+sums_B).

No max-subtraction is needed: logits*scale are bounded (~|40|) so exp stays
comfortably inside fp32/bf16 range.

Layouts are pre-transposed on the host so every matmul contraction dim is the
SBUF partition dim:
  QT[e, q] / KT[e, k]  -> scores^T[k, q] = sum_e KT^T QT  (k = psum partition)
  V[k, e]              -> O[q, e] = sum_k expS^T[q,k] V[k,e]
Sum-of-exp rides the same stationary operand with a ones[128,1] moving operand.

Attention runs over 512-wide query groups G (queries [512G, 512G+512)):
local key chunk u < 2G is fully valid for the whole group, chunk u == 2G
needs the triangular mask on the first 256 queries only, and chunk 2G+1
contributes only to the last 256 queries (triangular there). This gives a
uniform program across cores with zero wasted matmul work.
"""

import sys
import time

if "/opt/trn_rl_repo" not in sys.path:
    sys.path.insert(0, "/opt/trn_rl_repo")

import numpy as np
import ml_dtypes

B, S, D = 4, 2048, 1024
NCORES = 8
NCH = 8             # 128-row chunks of the contraction dim d
NQS = 4             # 512-wide q slices in Q projection
NKS = 2             # 512-wide slices over the 1024 core-local keys
NG = 4              # 512-wide query groups in attention
SCALE = 1.0 / 32.0  # 1/sqrt(D_OUT)

_CACHE = {}


def _build_module(repeat=1):
    key = ("nc", repeat)
    if key in _CACHE:
        return _CACHE[key]
    from contextlib import ExitStack
    import concourse.tile as tile
    from concourse import bacc, mybir

    f16 = mybir.dt.float16
    bf16 = mybir.dt.bfloat16
    f32 = mybir.dt.float32

    nc = bacc.Bacc("TRN2", target_bir_lowering=False, debug=False,
                   num_devices=NCORES)

    xT = nc.dram_tensor("xT", [D, S], f16, kind="ExternalInput").ap()
    xkT = nc.dram_tensor("xkT", [D, S // 2], f16, kind="ExternalInput").ap()
    wqT = nc.dram_tensor("wqT", [D, D], f16, kind="ExternalInput").ap()
    wkT = nc.dram_tensor("wkT", [D, D], f16, kind="ExternalInput").ap()
    wvT = nc.dram_tensor("wvT", [D, D], f16, kind="ExternalInput").ap()
    maskd = nc.dram_tensor("mask", [128, 256], bf16, kind="ExternalInput").ap()
    Od = nc.dram_tensor("O_part", [S, D], f32, kind="ExternalOutput").ap()
    sumd = nc.dram_tensor("sums", [128, 16], f32, kind="ExternalOutput").ap()

    with tile.TileContext(nc) as tc, ExitStack() as ctx:
        def pool(name, bufs, space="SBUF"):
            return ctx.enter_context(
                tc.tile_pool(name=name, bufs=bufs, space=space))

        p_wq = pool("wq", NCH)             # [128,1024] per e-block
        p_wk = pool("wk", NCH)             # [128,1024] per e-block
        p_wv = pool("wv", NKS)             # [128,4096] per es-slice
        p_xT = pool("xT", NQS)             # [128,4096] per qs-slice
        p_xkT = pool("xkT", NKS)           # [128,4096] per ks-slice
        p_QT = pool("QT", NCH)
        p_KT = pool("KT", NCH)
        p_V = pool("V", NCH)
        p_es = pool("es", 10)
        p_osb = pool("osb", 2)
        p_small = pool("small", 1)
        p_big = pool("pbig", 4, space="PSUM")    # 4 x 1 bank ([128,512] f32)
        p_st = pool("pst", 2, space="PSUM")      # 2 x 1 bank
        p_sum = pool("psum1", 2, space="PSUM")   # 2 x 1 bank

        # ---- input loads ----
        # Fine granularity only where it lets PE start early: wq e-block 0
        # and xT qs-slice 0 land as small tiles first; the rest come as big
        # chunks (fewer DMA descriptors -> less HWDGE occupancy).
        def dma_in(p, dram, ch, col0, width, dtype, nm):
            t = p.tile([128, width], dtype, name=nm, tag=nm.rstrip("0123456789_"))
            nc.sync.dma_start(t[:], dram[ch * 128:(ch + 1) * 128,
                                         col0:col0 + width])
            return t

        # Column-slice-major input loads: one strided DMA per logical slice
        # (dram [1024, w] -> sbuf [128, 8*w], chunk-major in the free dim).
        # 8x fewer DMA descriptors, and each load is exactly one matmul
        # group's dependency. Alternate the two HWDGE queues (sync/scalar).
        _dma_eng = [nc.sync, nc.scalar]
        _dma_i = [0]

        def dma_slice(p, dram, col0, width, dtype, nm):
            t = p.tile([128, NCH * width], dtype, name=nm,
                       tag=nm.rstrip("0123456789_"))
            src = dram[:, col0:col0 + width].rearrange(
                "(c p) w -> p c w", p=128)
            dst = t[:].rearrange("p (c w) -> p c w", c=NCH)
            _dma_eng[_dma_i[0] % 2].dma_start(dst, src)
            _dma_i[0] += 1
            return t

        # consumption order: QT(e0) needs wq_e[0] + xt_qs[0]; e0's later
        # groups need the other qs slices; then wq e1..7; then KT/V inputs.
        wq_e = [None] * NCH
        xt_qs = [None] * NQS
        wq_e[0] = dma_slice(p_wq, wqT, 0, 128, f16, "wqe_0")
        xt_qs[0] = dma_slice(p_xT, xT, 0, 512, f16, "xtq_0")
        for qs in range(1, NQS):
            xt_qs[qs] = dma_slice(p_xT, xT, qs * 512, 512, f16, f"xtq_{qs}")
        for e in range(1, NCH):
            wq_e[e] = dma_slice(p_wq, wqT, e * 128, 128, f16, f"wqe_{e}")
        wk_e = [dma_slice(p_wk, wkT, e * 128, 128, f16, f"wke_{e}")
                for e in range(NCH)]
        xk_ks = [dma_slice(p_xkT, xkT, ks * 512, 512, f16, f"xkq_{ks}")
                 for ks in range(NKS)]
        wv_es = [dma_slice(p_wv, wvT, es * 512, 512, f16, f"wvq_{es}")
                 for es in range(NKS)]

        def wq_slice(ch, e):
            return wq_e[e][:, ch * 128:(ch + 1) * 128]

        def xT_slice(ch, qs):
            return xt_qs[qs][:, ch * 512:(ch + 1) * 512]

        def wk_slice(ch, e):
            return wk_e[e][:, ch * 128:(ch + 1) * 128]

        def xk_slice(ch, ks):
            return xk_ks[ks][:, ch * 512:(ch + 1) * 512]

        def xk_stat(ch, kb):
            return xk_ks[kb // 4][:, ch * 512 + (kb % 4) * 128:
                                  ch * 512 + (kb % 4 + 1) * 128]

        def wv_slice(ch, es):
            return wv_es[es][:, ch * 512:(ch + 1) * 512]

        mask_sb = p_small.tile([128, 256], bf16, tag="mask")
        nc.sync.dma_start(mask_sb[:], maskd[:])
        ones_sb = p_small.tile([128, 1], bf16, tag="ones")
        nc.vector.memset(ones_sb[:], 1.0)
        sums_sb = p_small.tile([128, 16], f32, tag="sums")

        QT_t = [p_QT.tile([128, S], f16, tag="QT", name=f"QT{i}")
                for i in range(NCH)]
        KT_t = [p_KT.tile([128, S // 2], f16, tag="KT", name=f"KT{i}")
                for i in range(NCH)]
        V_t = [p_V.tile([128, D], bf16, tag="V", name=f"V{i}")
               for i in range(NCH)]

        for _rep in range(repeat):
            _emit_body(nc, mybir, p_big, p_st, p_sum, p_es, p_osb,
                       wq_slice, wk_slice, wv_slice, xT_slice, xk_slice, xk_stat,
                       QT_t, KT_t, V_t,
                       mask_sb, ones_sb, sums_sb, Od, sumd)

    nc.compile()
    _CACHE[key] = nc
    return nc


def _emit_body(nc, mybir, p_big, p_st, p_sum, p_es, p_osb,
               wq_slice, wk_slice, wv_slice, xT_slice, xk_slice, xk_stat,
               QT_t, KT_t, V_t,
               mask_sb, ones_sb, sums_sb, Od, sumd):
    f32 = mybir.dt.float32
    bf16 = mybir.dt.bfloat16
    Exp = mybir.ActivationFunctionType.Exp
    mm = nc.tensor.matmul

    # ---- projections ----
    # QT[e,q] += wqT[d,e].T @ xT[d,q]  (first: lowest marginal input rate)
    for e in range(NCH):
        for qs in range(NQS):
            ps = p_big.tile([128, 512], f32, tag="big", name=f"psq{e}_{qs}")
            for ch in range(NCH):
                mm(ps[:], wq_slice(ch, e), xT_slice(ch, qs),
                   start=(ch == 0), stop=(ch == NCH - 1))
            nc.vector.tensor_copy(QT_t[e][:, qs * 512:(qs + 1) * 512], ps[:])
    # KT[e,k] += wkT[d,e].T @ xkT[d,k]
    for e in range(NCH):
        for ks in range(NKS):
            ps = p_big.tile([128, 512], f32, tag="big", name=f"psk{e}_{ks}")
            for ch in range(NCH):
                mm(ps[:], wk_slice(ch, e), xk_slice(ch, ks),
                   start=(ch == 0), stop=(ch == NCH - 1))
            nc.vector.tensor_copy(KT_t[e][:, ks * 512:(ks + 1) * 512], ps[:])
    # V[k,e] += xkT[d,k].T @ wvT[d,e]
    for kb in range(NCH):
        for es in range(NKS):
            ps = p_big.tile([128, 512], f32, tag="big", name=f"psv{kb}_{es}")
            for ch in range(NCH):
                mm(ps[:], xk_stat(ch, kb), wv_slice(ch, es),
                   start=(ch == 0), stop=(ch == NCH - 1))
            nc.vector.tensor_copy(V_t[kb][:, es * 512:(es + 1) * 512], ps[:])

    # ---- attention over 512-wide query groups, largest group first so the
    # dependency tail (last ST -> last AV -> store) is as short as possible --
    for G in range(NG):
        es512 = []
        for u in range(2 * G + 1):
            st = p_st.tile([128, 512], f32, tag="st", name=f"st{G}_{u}")
            for ch in range(NCH):
                mm(st[:], KT_t[ch][:, u * 128:(u + 1) * 128],
                   QT_t[ch][:, G * 512:(G + 1) * 512],
                   start=(ch == 0), stop=(ch == NCH - 1))
            e_sb = p_es.tile([128, 512], bf16, tag="es", name=f"es{G}_{u}")
            nc.scalar.activation(e_sb[:], st[:], Exp, scale=SCALE)
            if u == 2 * G:
                nc.vector.tensor_mul(e_sb[:, 0:256], e_sb[:, 0:256],
                                     mask_sb[:])
            es512.append(e_sb)
        st2 = p_st.tile([128, 256], f32, tag="st", name=f"st2_{G}")
        for ch in range(NCH):
            mm(st2[:], KT_t[ch][:, (2 * G + 1) * 128:(2 * G + 2) * 128],
               QT_t[ch][:, G * 512 + 256:(G + 1) * 512],
               start=(ch == 0), stop=(ch == NCH - 1))
        e2 = p_es.tile([128, 256], bf16, tag="es", name=f"e2_{G}")
        nc.scalar.activation(e2[:], st2[:], Exp, scale=SCALE)
        nc.vector.tensor_mul(e2[:], e2[:], mask_sb[:])

        for tq in range(4):
            t_idx = 4 * G + tq
            late = tq >= 2          # second 256: chunk 2G+1 contributes
            av0 = p_big.tile([128, 512], f32, tag="big", name=f"av0_{t_idx}")
            av1 = p_big.tile([128, 512], f32, tag="big", name=f"av1_{t_idx}")
            sm = p_sum.tile([128, 1], f32, tag="sm", name=f"sm_{t_idx}")
            for u in range(2 * G + 1):
                stat = es512[u][:, tq * 128:(tq + 1) * 128]
                last = (u == 2 * G) and not late
                mm(av0[:], stat, V_t[u][:, 0:512], start=(u == 0), stop=last)
                mm(av1[:], stat, V_t[u][:, 512:1024], start=(u == 0),
                   stop=last)
                mm(sm[:], stat, ones_sb[:], start=(u == 0), stop=last)
            if late:
                stat = e2[:, (tq - 2) * 128:(tq - 1) * 128]
                u = 2 * G + 1
                mm(av0[:], stat, V_t[u][:, 0:512], start=False, stop=True)
                mm(av1[:], stat, V_t[u][:, 512:1024], start=False, stop=True)
                mm(sm[:], stat, ones_sb[:], start=False, stop=True)
            o_sb = p_osb.tile([128, 1024], f32, tag="o", name=f"o_{t_idx}")
            nc.vector.tensor_copy(o_sb[:, 0:512], av0[:])
            nc.sync.dma_start(Od[t_idx * 128:(t_idx + 1) * 128, 0:512],
                              o_sb[:, 0:512])
            nc.vector.tensor_copy(o_sb[:, 512:1024], av1[:])
            nc.sync.dma_start(Od[t_idx * 128:(t_idx + 1) * 128, 512:1024],
                              o_sb[:, 512:1024])
            nc.scalar.copy(sums_sb[:, t_idx:t_idx + 1], sm[:])
    nc.sync.dma_start(sumd[:], sums_sb[:])


def prepare_in_maps(x, W_query, W_key, W_value):
    x = np.asarray(x, dtype=np.float32)
    wqT = np.ascontiguousarray(np.asarray(W_query, np.float32).T).astype(np.float16)
    wkT = np.ascontiguousarray(np.asarray(W_key, np.float32).T).astype(np.float16)
    wvT = np.ascontiguousarray(np.asarray(W_value, np.float32).T).astype(np.float16)
    i = np.arange(128)[:, None]
    j = np.arange(256)[None, :]
    masks = [
        (i <= j).astype(ml_dtypes.bfloat16),          # parity 0 (even blocks)
        (128 + i <= j).astype(ml_dtypes.bfloat16),    # parity 1 (odd blocks)
    ]
    in_maps = []
    for c in range(NCORES):
        b, p = c // 2, c % 2
        xb = x[b]                                     # [S, D]
        xT = np.ascontiguousarray(xb.T).astype(np.float16)
        # rows of the core's key blocks: blocks 2u+p for u in 0..7
        xk = xb.reshape(16, 128, D)[p::2].reshape(S // 2, D)
        xkT = np.ascontiguousarray(xk.T).astype(np.float16)
        in_maps.append({
            "xT": xT, "xkT": xkT,
            "wqT": wqT, "wkT": wkT, "wvT": wvT,
            "mask": masks[p],
        })
    return in_maps


def merge_outputs(results):
    out = np.empty((B, S, D), dtype=np.float32)
    for b in range(B):
        r0, r1 = results[2 * b], results[2 * b + 1]
        num = r0["O_part"] + r1["O_part"]             # [S, D]
        # sums[p, t] holds q = t*128 + p
        s = (r0["sums"] + r1["sums"]).T.reshape(S)    # [S]
        out[b] = num / s[:, None]
    return out


def kernel(x, W_query, W_key, W_value):
    from concourse import bass_utils
    nc = _build_module()
    in_maps = prepare_in_maps(x, W_query, W_key, W_value)
    t0 = time.time()
    res = bass_utils.run_bass_kernel_spmd(
        nc, in_maps, core_ids=list(range(NCORES)))
    _CACHE["last_run_seconds"] = time.time() - t0
    return merge_outputs(res.results)


# revision 27
# speedup vs baseline: 1.2514x; 1.2514x over previous
"""Causal attention (B=4, S=2048, D=1024) on 8 trn2 NeuronCores.

Sharding: 2 cores per batch element, split over KEYS (interleaved 128-row
key blocks: core parity 0 takes even blocks, parity 1 takes odd blocks).
Each core computes Q for all 2048 positions, K/V for its 1024 keys, the
causally-masked exp-score block-band, the unnormalized partial output
O_part = sum_k exp(s_qk) v_k and the partial softmax denominator
sums_q = sum_k exp(s_qk).  The host merges: O = (O_A + O_B)/(sums_A+sums_B).

No max-subtraction is needed: logits*scale are bounded (~|40|) so exp stays
comfortably inside fp32/bf16 range.

Layouts are pre-transposed on the host so every matmul contraction dim is the
SBUF partition dim:
  QT[e, q] / KT[e, k]  -> scores^T[k, q] = sum_e KT^T QT  (k = psum partition)
  V[k, e]              -> O[q, e] = sum_k expS^T[q,k] V[k,e]
Sum-of-exp rides the same stationary operand with a ones[128,1] moving operand.

Attention runs over 512-wide query groups G (queries [512G, 512G+512)):
local key chunk u < 2G is fully valid for the whole group, chunk u == 2G
needs the triangular mask on the first 256 queries only, and chunk 2G+1
contributes only to the last 256 queries (triangular there). This gives a
uniform program across cores with zero wasted matmul work.
"""

import sys
import time

if "/opt/trn_rl_repo" not in sys.path:
    sys.path.insert(0, "/opt/trn_rl_repo")

import numpy as np
import ml_dtypes

B, S, D = 4, 2048, 1024
NCORES = 8
NCH = 8             # 128-row chunks of the contraction dim d
NQS = 4             # 512-wide q slices in Q projection
NKS = 2             # 512-wide slices over the 1024 core-local keys
NG = 4              # 512-wide query groups in attention
SCALE = 1.0 / 32.0  # 1/sqrt(D_OUT)

_CACHE = {}


def _build_module(repeat=1):
    key = ("nc", repeat)
    if key in _CACHE:
        return _CACHE[key]
    from contextlib import ExitStack
    import concourse.tile as tile
    from concourse import bacc, mybir

    f16 = mybir.dt.float16
    bf16 = mybir.dt.bfloat16
    f32 = mybir.dt.float32

    nc = bacc.Bacc("TRN2", target_bir_lowering=False, debug=False,
                   num_devices=NCORES)

    xT = nc.dram_tensor("xT", [D, S], f16, kind="ExternalInput").ap()
    xkT = nc.dram_tensor("xkT", [D, S // 2], f16, kind="ExternalInput").ap()
    wqT = nc.dram_tensor("wqT", [D, D // 2], f16, kind="ExternalInput").ap()
    wkT = nc.dram_tensor("wkT", [D, D], f16, kind="ExternalInput").ap()
    wvT = nc.dram_tensor("wvT", [D, D], f16, kind="ExternalInput").ap()
    maskd = nc.dram_tensor("mask", [128, 256], bf16, kind="ExternalInput").ap()
    Od = nc.dram_tensor("O_part", [S, D], f32, kind="ExternalOutput").ap()
    sumd = nc.dram_tensor("sums", [128, 16], f32, kind="ExternalOutput").ap()

    with tile.TileContext(nc) as tc, ExitStack() as ctx:
        def pool(name, bufs, space="SBUF"):
            return ctx.enter_context(
                tc.tile_pool(name=name, bufs=bufs, space=space))

        p_wq = pool("wq", NCH)             # [128,1024] per e-block
        p_wk = pool("wk", NCH)             # [128,1024] per e-block
        p_wv = pool("wv", NKS)             # [128,4096] per es-slice
        p_xT = pool("xT", NQS)             # [128,4096] per qs-slice
        p_xkT = pool("xkT", NKS)           # [128,4096] per ks-slice
        p_QT = pool("QT", NCH)
        p_QTl = pool("QTl", NCH // 2)
        p_KT = pool("KT", NCH)
        p_V = pool("V", NCH)
        p_es = pool("es", 10)
        p_osb = pool("osb", 2)
        p_small = pool("small", 1)
        p_dram = pool("dram", 1, space="DRAM")
        p_big = pool("pbig", 4, space="PSUM")    # 4 x 1 bank ([128,512] f32)
        p_st = pool("pst", 2, space="PSUM")      # 2 x 1 bank
        p_sum = pool("psum1", 2, space="PSUM")   # 2 x 1 bank

        # ---- input loads ----
        # Fine granularity only where it lets PE start early: wq e-block 0
        # and xT qs-slice 0 land as small tiles first; the rest come as big
        # chunks (fewer DMA descriptors -> less HWDGE occupancy).
        def dma_in(p, dram, ch, col0, width, dtype, nm):
            t = p.tile([128, width], dtype, name=nm, tag=nm.rstrip("0123456789_"))
            nc.sync.dma_start(t[:], dram[ch * 128:(ch + 1) * 128,
                                         col0:col0 + width])
            return t

        # Column-slice-major input loads: one strided DMA per logical slice
        # (dram [1024, w] -> sbuf [128, 8*w], chunk-major in the free dim).
        # 8x fewer DMA descriptors, and each load is exactly one matmul
        # group's dependency. Alternate the two HWDGE queues (sync/scalar).
        _dma_eng = [nc.sync, nc.scalar]
        _dma_i = [0]

        def dma_slice(p, dram, col0, width, dtype, nm):
            t = p.tile([128, NCH * width], dtype, name=nm,
                       tag=nm.rstrip("0123456789_"))
            src = dram[:, col0:col0 + width].rearrange(
                "(c p) w -> p c w", p=128)
            dst = t[:].rearrange("p (c w) -> p c w", c=NCH)
            _dma_eng[_dma_i[0] % 2].dma_start(dst, src)
            _dma_i[0] += 1
            return t

        # consumption order: QT(e0) needs wq_e[0] + xt_qs[0]; e0's later
        # groups need the other qs slices; then wq e1..7; then KT/V inputs.
        wq_e = [None] * (NCH // 2)
        xt_qs = [None] * NQS
        wq_e[0] = dma_slice(p_wq, wqT, 0, 128, f16, "wqe_0")
        xt_qs[0] = dma_slice(p_xT, xT, 0, 512, f16, "xtq_0")
        for qs in range(1, NQS):
            xt_qs[qs] = dma_slice(p_xT, xT, qs * 512, 512, f16, f"xtq_{qs}")
        for e in range(1, NCH // 2):
            wq_e[e] = dma_slice(p_wq, wqT, e * 128, 128, f16, f"wqe_{e}")
        wk_e = [dma_slice(p_wk, wkT, e * 128, 128, f16, f"wke_{e}")
                for e in range(NCH)]
        xk_ks = [dma_slice(p_xkT, xkT, ks * 512, 512, f16, f"xkq_{ks}")
                 for ks in range(NKS)]
        wv_es = [dma_slice(p_wv, wvT, es * 512, 512, f16, f"wvq_{es}")
                 for es in range(NKS)]

        def wq_slice(ch, e):
            return wq_e[e][:, ch * 128:(ch + 1) * 128]

        def xT_slice(ch, qs):
            return xt_qs[qs][:, ch * 512:(ch + 1) * 512]

        def wk_slice(ch, e):
            return wk_e[e][:, ch * 128:(ch + 1) * 128]

        def xk_slice(ch, ks):
            return xk_ks[ks][:, ch * 512:(ch + 1) * 512]

        def xk_stat(ch, kb):
            return xk_ks[kb // 4][:, ch * 512 + (kb % 4) * 128:
                                  ch * 512 + (kb % 4 + 1) * 128]

        def wv_slice(ch, es):
            return wv_es[es][:, ch * 512:(ch + 1) * 512]

        mask_sb = p_small.tile([128, 256], bf16, tag="mask")
        nc.sync.dma_start(mask_sb[:], maskd[:])
        ones_sb = p_small.tile([128, 1], bf16, tag="ones")
        nc.vector.memset(ones_sb[:], 1.0)
        sums_sb = p_small.tile([128, 16], f32, tag="sums")

        qt_half = p_dram.tile([D // 2, S], f16, tag="qth", name="qt_half")
        qt_full = p_dram.tile([D, S], f16, tag="qtf", name="qt_full")
        QTl_t = [p_QTl.tile([128, S], f16, tag="QTl", name=f"QTl{i}")
                 for i in range(NCH // 2)]
        QT_t = [p_QT.tile([128, S], f16, tag="QT", name=f"QT{i}")
                for i in range(NCH)]
        KT_t = [p_KT.tile([128, S // 2], f16, tag="KT", name=f"KT{i}")
                for i in range(NCH)]
        V_t = [p_V.tile([128, D], bf16, tag="V", name=f"V{i}")
               for i in range(NCH)]

        for _rep in range(repeat):
            _emit_body(nc, mybir, p_big, p_st, p_sum, p_es, p_osb,
                       wq_slice, wk_slice, wv_slice, xT_slice, xk_slice, xk_stat,
                       qt_half, qt_full, QTl_t, QT_t, KT_t, V_t,
                       mask_sb, ones_sb, sums_sb, Od, sumd)

    nc.compile()
    _CACHE[key] = nc
    return nc


def _emit_body(nc, mybir, p_big, p_st, p_sum, p_es, p_osb,
               wq_slice, wk_slice, wv_slice, xT_slice, xk_slice, xk_stat,
               qt_half, qt_full, QTl_t, QT_t, KT_t, V_t,
               mask_sb, ones_sb, sums_sb, Od, sumd):
    f32 = mybir.dt.float32
    bf16 = mybir.dt.bfloat16
    Exp = mybir.ActivationFunctionType.Exp
    mm = nc.tensor.matmul

    # ---- projections ----
    # QT[e,q] += wqT[d,e].T @ xT[d,q] for THIS CORE'S HALF of e only (the
    # host hands each core its parity's 512 columns of W_query^T); the two
    # halves are exchanged with a pairwise AllGather that overlaps the K/V
    # projections. Replica order == parity order, so qt_full rows land in
    # global e order.
    for e in range(NCH // 2):
        for qs in range(NQS):
            ps = p_big.tile([128, 512], f32, tag="big", name=f"psq{e}_{qs}")
            for ch in range(NCH):
                mm(ps[:], wq_slice(ch, e), xT_slice(ch, qs),
                   start=(ch == 0), stop=(ch == NCH - 1))
            nc.vector.tensor_copy(
                QTl_t[e][:, qs * 512:(qs + 1) * 512], ps[:])
    for e in range(NCH // 2):
        eng = nc.sync if e % 2 == 0 else nc.scalar
        eng.dma_start(qt_half[e * 128:(e + 1) * 128, :], QTl_t[e][:])
    nc.gpsimd.collective_compute(
        "AllGather", mybir.AluOpType.bypass,
        replica_groups=[[0, 1], [2, 3], [4, 5], [6, 7]],
        ins=[qt_half[:].opt()], outs=[qt_full[:].opt()],
    )
    for e in range(NCH):
        eng = nc.sync if e % 2 == 0 else nc.scalar
        eng.dma_start(QT_t[e][:], qt_full[e * 128:(e + 1) * 128, :])
    # KT[e,k] += wkT[d,e].T @ xkT[d,k]
    for e in range(NCH):
        for ks in range(NKS):
            ps = p_big.tile([128, 512], f32, tag="big", name=f"psk{e}_{ks}")
            for ch in range(NCH):
                mm(ps[:], wk_slice(ch, e), xk_slice(ch, ks),
                   start=(ch == 0), stop=(ch == NCH - 1))
            nc.vector.tensor_copy(KT_t[e][:, ks * 512:(ks + 1) * 512], ps[:])
    # V[k,e] += xkT[d,k].T @ wvT[d,e]
    for kb in range(NCH):
        for es in range(NKS):
            ps = p_big.tile([128, 512], f32, tag="big", name=f"psv{kb}_{es}")
            for ch in range(NCH):
                mm(ps[:], xk_stat(ch, kb), wv_slice(ch, es),
                   start=(ch == 0), stop=(ch == NCH - 1))
            nc.vector.tensor_copy(V_t[kb][:, es * 512:(es + 1) * 512], ps[:])

    # ---- attention over 512-wide query groups, largest group first so the
    # dependency tail (last ST -> last AV -> store) is as short as possible --
    for G in range(NG):
        es512 = []
        for u in range(2 * G + 1):
            st = p_st.tile([128, 512], f32, tag="st", name=f"st{G}_{u}")
            for ch in range(NCH):
                mm(st[:], KT_t[ch][:, u * 128:(u + 1) * 128],
                   QT_t[ch][:, G * 512:(G + 1) * 512],
                   start=(ch == 0), stop=(ch == NCH - 1))
            e_sb = p_es.tile([128, 512], bf16, tag="es", name=f"es{G}_{u}")
            nc.scalar.activation(e_sb[:], st[:], Exp, scale=SCALE)
            if u == 2 * G:
                nc.vector.tensor_mul(e_sb[:, 0:256], e_sb[:, 0:256],
                                     mask_sb[:])
            es512.append(e_sb)
        st2 = p_st.tile([128, 256], f32, tag="st", name=f"st2_{G}")
        for ch in range(NCH):
            mm(st2[:], KT_t[ch][:, (2 * G + 1) * 128:(2 * G + 2) * 128],
               QT_t[ch][:, G * 512 + 256:(G + 1) * 512],
               start=(ch == 0), stop=(ch == NCH - 1))
        e2 = p_es.tile([128, 256], bf16, tag="es", name=f"e2_{G}")
        nc.scalar.activation(e2[:], st2[:], Exp, scale=SCALE)
        nc.vector.tensor_mul(e2[:], e2[:], mask_sb[:])

        for tq in range(4):
            t_idx = 4 * G + tq
            late = tq >= 2          # second 256: chunk 2G+1 contributes
            av0 = p_big.tile([128, 512], f32, tag="big", name=f"av0_{t_idx}")
            av1 = p_big.tile([128, 512], f32, tag="big", name=f"av1_{t_idx}")
            sm = p_sum.tile([128, 1], f32, tag="sm", name=f"sm_{t_idx}")
            for u in range(2 * G + 1):
                stat = es512[u][:, tq * 128:(tq + 1) * 128]
                last = (u == 2 * G) and not late
                mm(av0[:], stat, V_t[u][:, 0:512], start=(u == 0), stop=last)
                mm(av1[:], stat, V_t[u][:, 512:1024], start=(u == 0),
                   stop=last)
                mm(sm[:], stat, ones_sb[:], start=(u == 0), stop=last)
            if late:
                stat = e2[:, (tq - 2) * 128:(tq - 1) * 128]
                u = 2 * G + 1
                mm(av0[:], stat, V_t[u][:, 0:512], start=False, stop=True)
                mm(av1[:], stat, V_t[u][:, 512:1024], start=False, stop=True)
                mm(sm[:], stat, ones_sb[:], start=False, stop=True)
            o_sb = p_osb.tile([128, 1024], f32, tag="o", name=f"o_{t_idx}")
            nc.vector.tensor_copy(o_sb[:, 0:512], av0[:])
            nc.sync.dma_start(Od[t_idx * 128:(t_idx + 1) * 128, 0:512],
                              o_sb[:, 0:512])
            nc.vector.tensor_copy(o_sb[:, 512:1024], av1[:])
            nc.sync.dma_start(Od[t_idx * 128:(t_idx + 1) * 128, 512:1024],
                              o_sb[:, 512:1024])
            nc.scalar.copy(sums_sb[:, t_idx:t_idx + 1], sm[:])
    nc.sync.dma_start(sumd[:], sums_sb[:])


def prepare_in_maps(x, W_query, W_key, W_value):
    x = np.asarray(x, dtype=np.float32)
    wqT_full = np.ascontiguousarray(np.asarray(W_query, np.float32).T).astype(np.float16)
    wq_halves = [np.ascontiguousarray(wqT_full[:, :512]),
                 np.ascontiguousarray(wqT_full[:, 512:])]
    wkT = np.ascontiguousarray(np.asarray(W_key, np.float32).T).astype(np.float16)
    wvT = np.ascontiguousarray(np.asarray(W_value, np.float32).T).astype(np.float16)
    i = np.arange(128)[:, None]
    j = np.arange(256)[None, :]
    masks = [
        (i <= j).astype(ml_dtypes.bfloat16),          # parity 0 (even blocks)
        (128 + i <= j).astype(ml_dtypes.bfloat16),    # parity 1 (odd blocks)
    ]
    in_maps = []
    for c in range(NCORES):
        b, p = c // 2, c % 2
        xb = x[b]                                     # [S, D]
        xT = np.ascontiguousarray(xb.T).astype(np.float16)
        # rows of the core's key blocks: blocks 2u+p for u in 0..7
        xk = xb.reshape(16, 128, D)[p::2].reshape(S // 2, D)
        xkT = np.ascontiguousarray(xk.T).astype(np.float16)
        in_maps.append({
            "xT": xT, "xkT": xkT,
            "wqT": wq_halves[p], "wkT": wkT, "wvT": wvT,
            "mask": masks[p],
        })
    return in_maps


def merge_outputs(results):
    out = np.empty((B, S, D), dtype=np.float32)
    for b in range(B):
        r0, r1 = results[2 * b], results[2 * b + 1]
        num = r0["O_part"] + r1["O_part"]             # [S, D]
        # sums[p, t] holds q = t*128 + p
        s = (r0["sums"] + r1["sums"]).T.reshape(S)    # [S]
        out[b] = num / s[:, None]
    return out


def kernel(x, W_query, W_key, W_value):
    from concourse import bass_utils
    nc = _build_module()
    in_maps = prepare_in_maps(x, W_query, W_key, W_value)
    t0 = time.time()
    res = bass_utils.run_bass_kernel_spmd(
        nc, in_maps, core_ids=list(range(NCORES)))
    _CACHE["last_run_seconds"] = time.time() - t0
    return merge_outputs(res.results)
